# revision 1
# baseline (speedup 1.0000x reference)
"""GraphTransformer 2-layer (TransformerConv x2) on 8 Trainium2 NeuronCores.

Sharding: destination-node partitioning with degree-sorted padded tiles.
  - Pad N=50000 -> N'=50176 (392 tiles of 128 nodes). Sort nodes by
    in-degree, bin-pack the tiles onto 8 cores (49 each, balancing slots).
  - Each core receives x^T in a per-core *rotated* node order (its own
    nodes first), computes the full layer-1 K|V table [N',512] on-device
    (replicated compute beats moving 100 MB), plus Q|S for its own nodes.
  - Attention per dst-tile: for neighbor-rank d an indirect DMA gathers
    the d-th neighbor's kv row for all 128 nodes (one row per partition).
    Padding slots point at row 0 and add -1e30 to the logit, so softmax
    kills them. Segment softmax is then plain free-dim reductions.
  - Layer-2 K|V [N',20] is computed from the local h chunk and AllGathered
    (4 MB on the wire instead of 51 MB of h).
All indices/degrees/tile shapes are baked in at build time from the actual
inputs. kernel() builds + runs the single-launch SPMD program and
unpermutes the output on the host.
"""

import numpy as np

N_CORES = 8
N = 50000
IN_DIM = 128
D1 = 256            # heads*hid layer1
H1, C1 = 8, 32
D2 = 10             # layer2 out channels (1 head)
P = 128
NEG = -1.0e30


def _plan(edge_index):
    src = np.asarray(edge_index[0], dtype=np.int64)
    dst = np.asarray(edge_index[1], dtype=np.int64)
    deg = np.bincount(dst, minlength=N)
    NP_ = ((N + N_CORES * P - 1) // (N_CORES * P)) * (N_CORES * P)  # 50176
    n_tiles = NP_ // P                                              # 392
    per_core = n_tiles // N_CORES                                   # 49

    degp = np.concatenate([deg, np.zeros(NP_ - N, np.int64)])
    order0 = np.argsort(degp, kind="stable")        # old(padded) ids, deg asc
    tile_of = order0.reshape(n_tiles, P)            # prelim tile -> old ids
    tile_D = degp[tile_of].max(axis=1)

    # bin-pack tiles onto cores: largest-first greedy with capacity
    t_order = np.argsort(-tile_D, kind="stable")
    loads = np.zeros(N_CORES, np.int64)
    counts = np.zeros(N_CORES, np.int64)
    assign = [[] for _ in range(N_CORES)]
    for t in t_order:
        open_cores = [c for c in range(N_CORES) if counts[c] < per_core]
        c = min(open_cores, key=lambda cc: (loads[cc], cc))
        assign[c].append(int(t))
        loads[c] += int(tile_D[t])
        counts[c] += 1
    for c in range(N_CORES):
        assign[c].sort(key=lambda t: int(tile_D[t]))

    final_tiles = [t for c in range(N_CORES) for t in assign[c]]
    perm = tile_of[final_tiles].reshape(-1)         # new id -> old(padded) id
    inv = np.empty(NP_, np.int64)
    inv[perm] = np.arange(NP_)

    Ds = degp[perm].reshape(n_tiles, P).max(axis=1).astype(np.int64)

    # per-(new)tile neighbor tables in NEW ids; pad idx=0, bias=NEG
    dst_new = inv[dst]
    src_new = inv[src]
    eo = np.argsort(dst_new, kind="stable")
    dst_s = dst_new[eo]
    src_s = src_new[eo]
    row_start = np.searchsorted(dst_s, np.arange(NP_))
    row_end = np.searchsorted(dst_s, np.arange(NP_) + 1)

    idx_tiles, bias_tiles = [], []
    for t in range(n_tiles):
        D = int(Ds[t])
        it = np.zeros((P, D), np.int64)
        bt = np.full((P, D), NEG, np.float32)
        for p in range(P):
            s, e = row_start[t * P + p], row_end[t * P + p]
            k = e - s
            it[p, :k] = src_s[s:e]
            bt[p, :k] = 0.0
        idx_tiles.append(it)
        bias_tiles.append(bt)

    return dict(NP=NP_, n_tiles=n_tiles, per_core=per_core, perm=perm,
                inv=inv, Ds=[int(d) for d in Ds], idx_tiles=idx_tiles,
                bias_tiles=bias_tiles)


def _build_program(NP_, per_core, Ds_pos, biases_zero, sim1=False):
    import concourse.bass as bass
    import concourse.mybir as mybir
    from concourse import bacc
    from concourse.tile import TileContext
    from concourse.masks import make_identity

    f32 = mybir.dt.float32
    i32 = mybir.dt.int32
    NOWN = per_core * P
    slots = sum(P * d for d in Ds_pos)
    Dmax = max(Ds_pos)
    slot_off = [0]
    for j in range(per_core):
        slot_off.append(slot_off[-1] + P * Ds_pos[j])

    nc = bacc.Bacc("TRN2", target_bir_lowering=False, debug=False,
                   num_devices=1 if sim1 else N_CORES)

    xT = nc.dram_tensor("xT", [IN_DIM, NP_], f32, kind="ExternalInput")
    w_kv1 = nc.dram_tensor("w_kv1", [IN_DIM, 2 * D1], f32, kind="ExternalInput")
    w_qs1 = nc.dram_tensor("w_qs1", [IN_DIM, 2 * D1], f32, kind="ExternalInput")
    w_kv2 = nc.dram_tensor("w_kv2", [D1, 2 * D2], f32, kind="ExternalInput")
    w_qs2 = nc.dram_tensor("w_qs2", [D1, 2 * D2], f32, kind="ExternalInput")
    b_kv1 = nc.dram_tensor("b_kv1", [1, 2 * D1], f32, kind="ExternalInput")
    b_qs1 = nc.dram_tensor("b_qs1", [1, 2 * D1], f32, kind="ExternalInput")
    b_kv2 = nc.dram_tensor("b_kv2", [1, 2 * D2], f32, kind="ExternalInput")
    b_qs2 = nc.dram_tensor("b_qs2", [1, 2 * D2], f32, kind="ExternalInput")
    idx1_f = nc.dram_tensor("idx1_f", [slots], i32, kind="ExternalInput")
    idx2_f = nc.dram_tensor("idx2_f", [slots], i32, kind="ExternalInput")
    bias_f = nc.dram_tensor("bias_f", [slots], f32, kind="ExternalInput")
    out_d = nc.dram_tensor("out", [NOWN, D2], f32, kind="ExternalOutput")

    kv1_t = nc.dram_tensor("kv1_t", [NP_, 2 * D1], f32)
    qs1_t = nc.dram_tensor("qs1_t", [NOWN, 2 * D1], f32)
    h_t = nc.dram_tensor("h_t", [NOWN, D1], f32)
    kv2_own = nc.dram_tensor("kv2_own", [NOWN, 2 * D2], f32)
    qs2_t = nc.dram_tensor("qs2_t", [NOWN, 2 * D2], f32)
    kv2_full = nc.dram_tensor("kv2_full", [NP_, 2 * D2], f32, addr_space="Shared")

    X = mybir.AxisListType.X
    MUL = mybir.AluOpType.mult
    ADD = mybir.AluOpType.add
    SUB = mybir.AluOpType.subtract
    EXP = mybir.ActivationFunctionType.Exp
    RELU = mybir.ActivationFunctionType.Relu

    with TileContext(nc) as tc:
        with tc.tile_pool(name="wpool", bufs=1) as wpool:
            w_kv1_s = wpool.tile([IN_DIM, 2 * D1], f32)
            nc.sync.dma_start(out=w_kv1_s[:], in_=w_kv1[:, :])
            w_qs1_s = wpool.tile([IN_DIM, 2 * D1], f32)
            nc.sync.dma_start(out=w_qs1_s[:], in_=w_qs1[:, :])
            w_kv2_s = wpool.tile([P, 2 * (2 * D2)], f32)
            nc.sync.dma_start(out=w_kv2_s[:, 0:2 * D2], in_=w_kv2[0:P, :])
            nc.sync.dma_start(out=w_kv2_s[:, 2 * D2:4 * D2], in_=w_kv2[P:2 * P, :])
            w_qs2_s = wpool.tile([P, 2 * (2 * D2)], f32)
            nc.sync.dma_start(out=w_qs2_s[:, 0:2 * D2], in_=w_qs2[0:P, :])
            nc.sync.dma_start(out=w_qs2_s[:, 2 * D2:4 * D2], in_=w_qs2[P:2 * P, :])
            if not biases_zero:
                ones1 = wpool.tile([1, P], f32)
                nc.vector.memset(ones1[:], 1.0)
                b_kv1_s = wpool.tile([1, 2 * D1], f32)
                nc.sync.dma_start(out=b_kv1_s[:], in_=b_kv1[:, :])
                b_qs1_s = wpool.tile([1, 2 * D1], f32)
                nc.sync.dma_start(out=b_qs1_s[:], in_=b_qs1[:, :])
                b_kv2_s = wpool.tile([1, 2 * D2], f32)
                nc.sync.dma_start(out=b_kv2_s[:], in_=b_kv2[:, :])
                b_qs2_s = wpool.tile([1, 2 * D2], f32)
                nc.sync.dma_start(out=b_qs2_s[:], in_=b_qs2[:, :])
            ident = wpool.tile([P, P], f32)
            make_identity(nc, ident[:])

            # ================= P1: layer-1 projections =================
            with tc.tile_pool(name="p1x", bufs=3) as p1x, \
                 tc.tile_pool(name="p1ps", bufs=4, space="PSUM") as p1ps, \
                 tc.tile_pool(name="p1o", bufs=4) as p1o:
                XB = 512
                for blk in range(NP_ // XB):
                    xT_s = p1x.tile([P, XB], f32, tag="xT")
                    nc.sync.dma_start(out=xT_s[:],
                                      in_=xT[:, blk * XB:(blk + 1) * XB])
                    for jj in range(XB // P):
                        t = blk * (XB // P) + jj
                        lhsT = xT_s[:, jj * P:(jj + 1) * P]
                        ps = p1ps.tile([P, 2 * D1], f32, tag="ps")
                        nc.tensor.matmul(out=ps[:], lhsT=lhsT, rhs=w_kv1_s[:],
                                         start=True, stop=biases_zero)
                        if not biases_zero:
                            nc.tensor.matmul(out=ps[:], lhsT=ones1[:],
                                             rhs=b_kv1_s[:], start=False, stop=True)
                        kv_o = p1o.tile([P, 2 * D1], f32, tag="kv")
                        nc.any.tensor_copy(out=kv_o[:], in_=ps[:])
                        nc.sync.dma_start(out=kv1_t[t * P:(t + 1) * P, :],
                                          in_=kv_o[:])
                        if t < per_core:   # own nodes (rotated order)
                            ps2 = p1ps.tile([P, 2 * D1], f32, tag="ps")
                            nc.tensor.matmul(out=ps2[:], lhsT=lhsT, rhs=w_qs1_s[:],
                                             start=True, stop=biases_zero)
                            if not biases_zero:
                                nc.tensor.matmul(out=ps2[:], lhsT=ones1[:],
                                                 rhs=b_qs1_s[:], start=False,
                                                 stop=True)
                            qs_o = p1o.tile([P, 2 * D1], f32, tag="kv")
                            nc.any.tensor_copy(out=qs_o[:], in_=ps2[:])
                            nc.sync.dma_start(out=qs1_t[t * P:(t + 1) * P, :],
                                              in_=qs_o[:])

            # ========== P2+P3: layer-1 attention + layer-2 projections ==========
            with tc.tile_pool(name="kvb", bufs=2) as kvb, \
                 tc.tile_pool(name="meta", bufs=2) as meta, \
                 tc.tile_pool(name="small", bufs=2) as small, \
                 tc.tile_pool(name="hps", bufs=2, space="PSUM") as hps, \
                 tc.tile_pool(name="houtp", bufs=2) as houtp:
                for j in range(per_core):
                    D = Ds_pos[j]
                    qs_s = meta.tile([P, 2 * D1], f32, tag="qs")
                    nc.sync.dma_start(out=qs_s[:],
                                      in_=qs1_t[j * P:(j + 1) * P, :])
                    idx_s = meta.tile([P, Dmax], i32, tag="idx")
                    nc.sync.dma_start(
                        out=idx_s[:, 0:D],
                        in_=idx1_f[slot_off[j]:slot_off[j + 1]]
                            .rearrange("(p d) -> p d", d=D))
                    bias_s = meta.tile([P, Dmax], f32, tag="bias")
                    nc.sync.dma_start(
                        out=bias_s[:, 0:D],
                        in_=bias_f[slot_off[j]:slot_off[j + 1]]
                            .rearrange("(p d) -> p d", d=D))
                    kv_s = kvb.tile([P, Dmax * 2 * D1], f32, tag="kv")
                    for d in range(D):
                        nc.gpsimd.indirect_dma_start(
                            out=kv_s[:, d * 2 * D1:(d + 1) * 2 * D1],
                            out_offset=None,
                            in_=kv1_t[:, :],
                            in_offset=bass.IndirectOffsetOnAxis(
                                ap=idx_s[:, d:d + 1], axis=0))
                    kv3 = kv_s[:].rearrange("p (d f) -> p d f", d=Dmax)
                    nc.vector.tensor_tensor(
                        out=kv3[:, 0:D, 0:D1], in0=kv3[:, 0:D, 0:D1],
                        in1=qs_s[:, 0:D1].unsqueeze(1).to_broadcast([P, D, D1]),
                        op=MUL)
                    lg = small.tile([P, Dmax * H1], f32, tag="lg")
                    lgv = lg[:, 0:D * H1].rearrange("p (d h) -> p d h", d=D)
                    nc.vector.reduce_sum(
                        out=lgv,
                        in_=kv3[:, 0:D, 0:D1].rearrange(
                            "p d (h c) -> p d h c", h=H1),
                        axis=X)
                    nc.vector.tensor_tensor(
                        out=lgv, in0=lgv,
                        in1=bias_s[:, 0:D].unsqueeze(2).to_broadcast([P, D, H1]),
                        op=ADD)
                    mx = small.tile([P, H1], f32, tag="mx")
                    nc.vector.reduce_max(
                        out=mx[:],
                        in_=lg[:, 0:D * H1].rearrange("p (d h) -> p h d", d=D),
                        axis=X)
                    nc.vector.tensor_tensor(
                        out=lgv, in0=lgv,
                        in1=mx[:].unsqueeze(1).to_broadcast([P, D, H1]),
                        op=SUB)
                    nc.scalar.activation(out=lg[:, 0:D * H1], in_=lg[:, 0:D * H1],
                                         func=EXP)
                    sm = small.tile([P, H1], f32, tag="sm")
                    nc.vector.reduce_sum(
                        out=sm[:],
                        in_=lg[:, 0:D * H1].rearrange("p (d h) -> p h d", d=D),
                        axis=X)
                    nc.vector.tensor_scalar_add(out=sm[:], in0=sm[:], scalar1=1e-16)
                    rc = small.tile([P, H1], f32, tag="rc")
                    nc.vector.reciprocal(out=rc[:], in_=sm[:])
                    nc.vector.tensor_tensor(
                        out=kv3[:, 0:D, D1:2 * D1].rearrange(
                            "p d (h c) -> p d h c", h=H1),
                        in0=kv3[:, 0:D, D1:2 * D1].rearrange(
                            "p d (h c) -> p d h c", h=H1),
                        in1=lgv.unsqueeze(3).to_broadcast([P, D, H1, C1]),
                        op=MUL)
                    att = houtp.tile([P, D1], f32, tag="att")
                    nc.vector.reduce_sum(
                        out=att[:],
                        in_=kv3[:, 0:D, D1:2 * D1].transpose([0, 2, 1]),
                        axis=X)
                    nc.vector.tensor_tensor(
                        out=att[:].rearrange("p (h c) -> p h c", h=H1),
                        in0=att[:].rearrange("p (h c) -> p h c", h=H1),
                        in1=rc[:].unsqueeze(2).to_broadcast([P, H1, C1]),
                        op=MUL)
                    nc.vector.tensor_add(out=att[:], in0=att[:],
                                         in1=qs_s[:, D1:2 * D1])
                    # ELU: h = relu(z) + exp(min(z,0)) - 1
                    zmin = houtp.tile([P, D1], f32, tag="zmin")
                    nc.vector.tensor_scalar_min(out=zmin[:], in0=att[:],
                                                scalar1=0.0)
                    nc.scalar.activation(out=zmin[:], in_=zmin[:], func=EXP)
                    h_s = houtp.tile([P, D1], f32, tag="h")
                    nc.scalar.activation(out=h_s[:], in_=att[:], func=RELU)
                    nc.vector.tensor_add(out=h_s[:], in0=h_s[:], in1=zmin[:])
                    nc.vector.tensor_scalar_add(out=h_s[:], in0=h_s[:],
                                                scalar1=-1.0)
                    nc.sync.dma_start(out=h_t[j * P:(j + 1) * P, :], in_=h_s[:])

                    # ---- layer-2 projections for this tile ----
                    hT0 = hps.tile([P, P], f32, tag="hT")
                    nc.tensor.transpose(out=hT0[:], in_=h_s[:, 0:P],
                                        identity=ident[:])
                    hT0s = houtp.tile([P, P], f32, tag="hT0s")
                    nc.any.tensor_copy(out=hT0s[:], in_=hT0[:])
                    hT1 = hps.tile([P, P], f32, tag="hT")
                    nc.tensor.transpose(out=hT1[:], in_=h_s[:, P:2 * P],
                                        identity=ident[:])
                    hT1s = houtp.tile([P, P], f32, tag="hT1s")
                    nc.any.tensor_copy(out=hT1s[:], in_=hT1[:])
                    for wi, (wt, dest) in enumerate(((w_kv2_s, kv2_own),
                                                     (w_qs2_s, qs2_t))):
                        ps = hps.tile([P, 2 * D2], f32, tag="ps2")
                        nc.tensor.matmul(out=ps[:], lhsT=hT0s[:],
                                         rhs=wt[:, 0:2 * D2],
                                         start=True, stop=False)
                        nc.tensor.matmul(out=ps[:], lhsT=hT1s[:],
                                         rhs=wt[:, 2 * D2:4 * D2],
                                         start=False, stop=biases_zero)
                        if not biases_zero:
                            bs = b_kv2_s if wi == 0 else b_qs2_s
                            nc.tensor.matmul(out=ps[:], lhsT=ones1[:], rhs=bs[:],
                                             start=False, stop=True)
                        os_ = houtp.tile([P, 2 * D2], f32, tag="os2")
                        nc.any.tensor_copy(out=os_[:], in_=ps[:])
                        nc.sync.dma_start(out=dest[j * P:(j + 1) * P, :],
                                          in_=os_[:])

            # ================= P4: AllGather kv2 =================
            if sim1:
                for c in range(N_CORES):
                    nc.sync.dma_start(
                        out=kv2_full[c * NOWN:(c + 1) * NOWN, :],
                        in_=kv2_own[:, :])
            else:
                nc.gpsimd.collective_compute(
                    "AllGather", mybir.AluOpType.bypass,
                    replica_groups=[list(range(N_CORES))],
                    ins=[kv2_own.ap().opt()],
                    outs=[kv2_full.ap().opt()],
                )

            # ================= P5: layer-2 attention =================
            with tc.tile_pool(name="kvb2", bufs=2) as kvb2, \
                 tc.tile_pool(name="meta2", bufs=2) as meta2, \
                 tc.tile_pool(name="small2", bufs=2) as small2, \
                 tc.tile_pool(name="outp", bufs=2) as outp:
                for j in range(per_core):
                    D = Ds_pos[j]
                    qs_s = meta2.tile([P, 2 * D2], f32, tag="qs2")
                    nc.sync.dma_start(out=qs_s[:],
                                      in_=qs2_t[j * P:(j + 1) * P, :])
                    idx_s = meta2.tile([P, Dmax], i32, tag="idx2")
                    nc.sync.dma_start(
                        out=idx_s[:, 0:D],
                        in_=idx2_f[slot_off[j]:slot_off[j + 1]]
                            .rearrange("(p d) -> p d", d=D))
                    bias_s = meta2.tile([P, Dmax], f32, tag="bias2")
                    nc.sync.dma_start(
                        out=bias_s[:, 0:D],
                        in_=bias_f[slot_off[j]:slot_off[j + 1]]
                            .rearrange("(p d) -> p d", d=D))
                    kv_s = kvb2.tile([P, Dmax * 2 * D2], f32, tag="kv2")
                    for d in range(D):
                        nc.gpsimd.indirect_dma_start(
                            out=kv_s[:, d * 2 * D2:(d + 1) * 2 * D2],
                            out_offset=None,
                            in_=kv2_full[:, :],
                            in_offset=bass.IndirectOffsetOnAxis(
                                ap=idx_s[:, d:d + 1], axis=0))
                    kv3 = kv_s[:].rearrange("p (d f) -> p d f", d=Dmax)
                    nc.vector.tensor_tensor(
                        out=kv3[:, 0:D, 0:D2], in0=kv3[:, 0:D, 0:D2],
                        in1=qs_s[:, 0:D2].unsqueeze(1).to_broadcast([P, D, D2]),
                        op=MUL)
                    lg = small2.tile([P, Dmax], f32, tag="lg2")
                    nc.vector.reduce_sum(out=lg[:, 0:D], in_=kv3[:, 0:D, 0:D2],
                                         axis=X)
                    nc.vector.tensor_add(out=lg[:, 0:D], in0=lg[:, 0:D],
                                         in1=bias_s[:, 0:D])
                    mx = small2.tile([P, 1], f32, tag="mx2")
                    nc.vector.reduce_max(out=mx[:], in_=lg[:, 0:D], axis=X)
                    nc.vector.tensor_tensor(out=lg[:, 0:D], in0=lg[:, 0:D],
                                            in1=mx[:].to_broadcast([P, D]),
                                            op=SUB)
                    nc.scalar.activation(out=lg[:, 0:D], in_=lg[:, 0:D], func=EXP)
                    sm = small2.tile([P, 1], f32, tag="sm2")
                    nc.vector.reduce_sum(out=sm[:], in_=lg[:, 0:D], axis=X)
                    nc.vector.tensor_scalar_add(out=sm[:], in0=sm[:],
                                                scalar1=1e-16)
                    rc = small2.tile([P, 1], f32, tag="rc2")
                    nc.vector.reciprocal(out=rc[:], in_=sm[:])
                    nc.vector.tensor_tensor(
                        out=kv3[:, 0:D, D2:2 * D2], in0=kv3[:, 0:D, D2:2 * D2],
                        in1=lg[:, 0:D].unsqueeze(2).to_broadcast([P, D, D2]),
                        op=MUL)
                    att = outp.tile([P, D2], f32, tag="att2")
                    nc.vector.reduce_sum(
                        out=att[:],
                        in_=kv3[:, 0:D, D2:2 * D2].transpose([0, 2, 1]),
                        axis=X)
                    nc.vector.tensor_tensor(out=att[:], in0=att[:],
                                            in1=rc[:].to_broadcast([P, D2]),
                                            op=MUL)
                    nc.vector.tensor_add(out=att[:], in0=att[:],
                                         in1=qs_s[:, D2:2 * D2])
                    nc.sync.dma_start(out=out_d[j * P:(j + 1) * P, :],
                                      in_=att[:])

    nc.compile()
    return nc


_CACHE = {}


def _get_program(NP_, per_core, Ds_pos, biases_zero):
    key = (NP_, per_core, tuple(Ds_pos), biases_zero)
    if key not in _CACHE:
        _CACHE[key] = _build_program(NP_, per_core, Ds_pos, biases_zero)
    return _CACHE[key]


def kernel(**inputs):
    from concourse.bass_utils import run_bass_kernel_spmd

    x = np.asarray(inputs["x"], np.float32)
    edge_index = np.asarray(inputs["edge_index"])
    plan = _plan(edge_index)
    NP_ = plan["NP"]
    per_core = plan["per_core"]
    Ds = plan["Ds"]
    NOWN = per_core * P

    # position-aligned degrees (SPMD: one program for all cores)
    Ds_pos = [max(Ds[c * per_core + j] for c in range(N_CORES))
              for j in range(per_core)]

    s1 = 1.0 / np.sqrt(np.float32(C1))
    s2 = 1.0 / np.sqrt(np.float32(D2))
    w_kv1 = np.ascontiguousarray(
        np.concatenate([inputs["w1k"], inputs["w1v"]], axis=1), np.float32)
    w_qs1 = np.ascontiguousarray(
        np.concatenate([np.asarray(inputs["w1q"]) * s1, inputs["w1s"]], axis=1),
        np.float32)
    w_kv2 = np.ascontiguousarray(
        np.concatenate([inputs["w2k"], inputs["w2v"]], axis=1), np.float32)
    w_qs2 = np.ascontiguousarray(
        np.concatenate([np.asarray(inputs["w2q"]) * s2, inputs["w2s"]], axis=1),
        np.float32)
    b_kv1 = np.ascontiguousarray(
        np.concatenate([inputs["b1k"], inputs["b1v"]])[None], np.float32)
    b_qs1 = np.ascontiguousarray(
        np.concatenate([np.asarray(inputs["b1q"]) * s1, inputs["b1s"]])[None],
        np.float32)
    b_kv2 = np.ascontiguousarray(
        np.concatenate([inputs["b2k"], inputs["b2v"]])[None], np.float32)
    b_qs2 = np.ascontiguousarray(
        np.concatenate([np.asarray(inputs["b2q"]) * s2, inputs["b2s"]])[None],
        np.float32)
    biases_zero = all(not np.any(b) for b in (b_kv1, b_qs1, b_kv2, b_qs2))

    nc = _get_program(NP_, per_core, Ds_pos, biases_zero)

    xpad = np.concatenate([x, np.zeros((NP_ - N, IN_DIM), np.float32)])
    x_new = xpad[plan["perm"]]
    xT_new = np.ascontiguousarray(x_new.T)

    in_maps = []
    for c in range(N_CORES):
        own0 = c * NOWN
        rot = np.concatenate([np.arange(own0, own0 + NOWN),
                              np.arange(0, own0),
                              np.arange(own0 + NOWN, NP_)])
        inv_rot = np.empty(NP_, np.int64)
        inv_rot[rot] = np.arange(NP_)
        xT_c = np.ascontiguousarray(xT_new[:, rot])
        idx1_list, idx2_list, bias_list = [], [], []
        for j in range(per_core):
            t_new = c * per_core + j
            D = Ds[t_new]
            Dp = Ds_pos[j]
            it = plan["idx_tiles"][t_new]       # [P, D] new ids
            bt = plan["bias_tiles"][t_new]
            i1 = np.zeros((P, Dp), np.int32)
            i2 = np.zeros((P, Dp), np.int32)
            bp = np.full((P, Dp), NEG, np.float32)
            i1[:, :D] = inv_rot[it]             # rotated ids (layer-1 table)
            i2[:, :D] = it                      # global new ids (layer-2 table)
            bp[:, :D] = bt
            idx1_list.append(i1.reshape(-1))
            idx2_list.append(i2.reshape(-1))
            bias_list.append(bp.reshape(-1))
        in_maps.append(dict(
            xT=xT_c,
            w_kv1=w_kv1, w_qs1=w_qs1, w_kv2=w_kv2, w_qs2=w_qs2,
            b_kv1=b_kv1, b_qs1=b_qs1, b_kv2=b_kv2, b_qs2=b_qs2,
            idx1_f=np.concatenate(idx1_list),
            idx2_f=np.concatenate(idx2_list),
            bias_f=np.concatenate(bias_list),
        ))

    res = run_bass_kernel_spmd(nc, in_maps, core_ids=list(range(N_CORES)))
    kernel.last_results = res

    out_new = np.concatenate([np.asarray(res.results[c]["out"])
                              for c in range(N_CORES)])
    mask = plan["perm"] < N
    out = np.empty((N, D2), np.float32)
    out[plan["perm"][mask]] = out_new[mask]
    return out

